# revision 10
# baseline (speedup 1.0000x reference)
"""AerialPatchSampler Trainium2 kernel.

Samples N rotated/scaled/translated 64x64 patches from a (C=64, 512, 512)
aerial feature image with bilinear interpolation (grid_sample semantics,
align_corners=False, zeros padding + validity mask).

Sharding: 8 cores; core k handles batch b = k//4 and hypotheses
n in [32*(k%4), 32*(k%4)+32).  Each core receives its batch's full image.

Per-core plan:
  Phase A: build qimg[(Hp*Wp), 2*C] in DRAM: qimg[(y*Wp+x)] holds the
           channel vectors of padded rows y and y+1 at column x
           (zero border = grid_sample zero padding).  Built with PE
           transposes of (row-pair, channel) tiles.
  Phase B: per patch: compute grid coords + bilinear weights on DVE/ACT;
           one indirect-DMA index per output pixel reads 256 contiguous
           floats covering all 4 bilinear corners (HW indirect DMA
           semantics: one index per destination partition, reading the
           dest row length from idx * src_inner_dim); combine with
           per-pixel weights (channel-broadcast via stride-0 APs);
           PE-transpose back to channel-major; store contiguously.
"""

import sys

for _p in ("/opt/trn_rl_repo", "/root/.axon_site/_ro/trn_rl_repo"):
    if _p not in sys.path:
        sys.path.insert(0, _p)

import numpy as np

import concourse.bass as bass
import concourse.tile as tile
from concourse import bacc, mybir
from concourse.bass import AP
from concourse.masks import make_identity

F32 = mybir.dt.float32
I32 = mybir.dt.int32
ALU = mybir.AluOpType
ACTF = mybir.ActivationFunctionType

B, C, H, W = 2, 64, 512, 512
N = 128
HB, WB = 64, 64
NCORES = 8
NP = N // (NCORES // B)  # 32 patches per core
Hp, Wp = H + 2, W + 2  # 514, zero-padded
PV = Hp * Wp
PIX = HB * WB  # 4096
MAGIC = 12582912.0  # 1.5 * 2^23: RNE-to-int trick, ULP 1.0 for |t| < 2^22


def _ap(base: AP, extra_off: int, dims) -> AP:
    return AP(base.tensor, base.offset + extra_off, [list(d) for d in dims])


def build_program():
    nc = bacc.Bacc(
        "TRN2",
        target_bir_lowering=False,
        debug=False,
        enable_asserts=False,
    )
    img = nc.dram_tensor("img", [C, H, W], F32, kind="ExternalInput").ap()
    pose = nc.dram_tensor("pose", [NP, 3], F32, kind="ExternalInput").ap()
    osc = nc.dram_tensor("osc", [1, 1], F32, kind="ExternalInput").ap()
    out = nc.dram_tensor("out", [NP, C, PIX], F32, kind="ExternalOutput").ap()
    qimg = nc.dram_tensor("qimg", [PV, 2 * C], F32, kind="Internal").ap()

    with tile.TileContext(nc, trace_sim=False) as tc:
        with tc.tile_pool(name="const", bufs=1) as cpool:
            ident = cpool.tile([128, 128], F32)
            make_identity(nc, ident[:])

            zt = cpool.tile([128, 257], F32)
            nc.vector.memset(zt[:], 0.0)

            # ---- params broadcast to all partitions ----
            u_bc = cpool.tile([128, NP], F32)
            v_bc = cpool.tile([128, NP], F32)
            th_bc = cpool.tile([128, NP], F32)
            nc.sync.dma_start(u_bc[:], _ap(pose, 0, [[0, 128], [3, NP]]))
            nc.sync.dma_start(v_bc[:], _ap(pose, 1, [[0, 128], [3, NP]]))
            nc.sync.dma_start(th_bc[:], _ap(pose, 2, [[0, 128], [3, NP]]))
            os_bc = cpool.tile([128, 1], F32)
            nc.sync.dma_start(os_bc[:], _ap(osc, 0, [[0, 128], [1, 1]]))

            # cos(-th) = sin(pi/2 - |th|);  sin(-th) = sin(th * -1)
            zbias = cpool.tile([128, 1], F32)
            nc.vector.memset(zbias[:], 0.0)
            pibias = cpool.tile([128, 1], F32)
            nc.vector.memset(pibias[:], 1.5707963267948966)
            sin_bc = cpool.tile([128, NP], F32)
            cos_bc = cpool.tile([128, NP], F32)
            abs_th = cpool.tile([128, NP], F32)
            nc.scalar.activation(
                sin_bc[:], th_bc[:], ACTF.Sin, scale=-1.0, bias=zbias[:, 0:1]
            )
            nc.scalar.activation(abs_th[:], th_bc[:], ACTF.Abs, bias=zbias[:, 0:1])
            nc.scalar.activation(
                cos_bc[:], abs_th[:], ACTF.Sin, scale=-1.0, bias=pibias[:, 0:1]
            )

            # ---- base grids ----
            # pixel p = if*128 + part; i = 2*if + part//64; j = part%64
            pi32 = cpool.tile([128, 1], I32)
            nc.gpsimd.iota(pi32[:], pattern=[[0, 1]], base=0, channel_multiplier=1)
            j32 = cpool.tile([128, 1], I32)
            nc.vector.tensor_scalar(j32[:], pi32[:], 63, None, ALU.bitwise_and)
            pd64 = cpool.tile([128, 1], I32)
            nc.vector.tensor_scalar(pd64[:], pi32[:], 6, None, ALU.arith_shift_right)
            if2 = cpool.tile([128, 32], I32)
            nc.gpsimd.iota(if2[:], pattern=[[2, 32]], base=0, channel_multiplier=0)

            jf = cpool.tile([128, 1], F32)
            nc.vector.tensor_copy(jf[:], j32[:])
            pdf = cpool.tile([128, 1], F32)
            nc.vector.tensor_copy(pdf[:], pd64[:])
            i_f = cpool.tile([128, 32], F32)
            nc.vector.tensor_copy(i_f[:], if2[:])
            nc.vector.tensor_scalar(i_f[:], i_f[:], pdf[:, 0:1], None, ALU.add)

            # gu0 = (63 - i) * os ;  gv0 = (j - 32) * os
            gu0 = cpool.tile([128, 32], F32)
            nc.vector.tensor_scalar(gu0[:], i_f[:], -1.0, 63.0, ALU.mult, ALU.add)
            nc.vector.tensor_scalar(gu0[:], gu0[:], os_bc[:, 0:1], None, ALU.mult)
            gv0 = cpool.tile([128, 1], F32)
            nc.vector.tensor_scalar(
                gv0[:], jf[:], 32.0, os_bc[:, 0:1], ALU.subtract, ALU.mult
            )

            # ============ Phase A: build double-row channel-last image =====
            # qimg[y*Wp + x] = [chan(padrow y, x), chan(padrow y+1, x)]
            # zero fills:
            #  - column pads: slots (y, Wp-1) and (y+1, 0) are contiguous
            #  - q row 0 low half (pad row 0); q row H low... (handled below)
            ztap = zt[:]

            def zfill(dst_off, nblk, blk_stride, blk_len):
                done = 0
                while done < nblk:
                    cnt = min(128, nblk - done)
                    nc.scalar.dma_start(
                        _ap(
                            qimg,
                            dst_off + done * blk_stride,
                            [[blk_stride, cnt], [1, blk_len]],
                        ),
                        _ap(ztap, 0, [[257, cnt], [1, blk_len]]),
                    )
                    done += cnt

            QR = 2 * C  # 128 elements per q slot
            # col pads for q rows 0..512: slot (y, 513) + slot (y+1, 0)
            zfill((Wp - 1) * QR, H + 1, Wp * QR, 2 * QR)
            zfill(0, 1, QR, QR)  # slot (0, 0)
            # q row 0 low halves (pad image row): cols 1..512
            zfill(QR, W, QR, C)
            # q row 512 high halves (pad image row): cols 1..512
            zfill((H * Wp + 1) * QR + C, W, QR, C)
            # q row 513: never gathered, but keep DRAM finite
            zfill((H + 1) * Wp * QR, Wp, QR, QR)

            with (
                tc.tile_pool(name="lpool", bufs=3) as lpool,
                tc.tile_pool(name="papsum", bufs=4, space="PSUM") as papsum,
                tc.tile_pool(name="spool", bufs=3) as spool,
            ):
                for yi in range(1, H):  # q row yi = (img rows yi-1, yi)
                    lt = lpool.tile([128, 512], F32)
                    nc.sync.dma_start(
                        lt[:],
                        _ap(img, (yi - 1) * W, [[W, 2], [H * W, C], [1, W]]),
                    )
                    bt = papsum.tile([128, 512], F32)
                    for k in range(4):
                        nc.tensor.transpose(
                            out=bt[:, 128 * k : 128 * (k + 1)],
                            in_=lt[:, 128 * k : 128 * (k + 1)],
                            identity=ident[:],
                        )
                    st = spool.tile([128, 512], F32)
                    nc.vector.tensor_copy(st[:], bt[:])
                    # st[x, k*128 + y2*64 + c] == q slot payload (y2 in order)
                    sap = st[:]
                    nc.scalar.dma_start(
                        _ap(
                            qimg,
                            (yi * Wp + 1) * QR,
                            [[QR, 128], [128 * QR, 4], [1, QR]],
                        ),
                        _ap(sap, 0, [sap.ap[0], [128, 4], [1, QR]]),
                    )
                    if yi == 1:
                        # q row 0 high half = img row 0 = st y2=0 cols
                        nc.scalar.dma_start(
                            _ap(qimg, QR + C, [[QR, 128], [128 * QR, 4], [1, C]]),
                            _ap(sap, 0, [sap.ap[0], [128, 4], [1, C]]),
                        )
                    if yi == H - 1:
                        # q row 512 low half = img row 511 = st y2=1 cols
                        nc.scalar.dma_start(
                            _ap(
                                qimg,
                                (H * Wp + 1) * QR,
                                [[QR, 128], [128 * QR, 4], [1, C]],
                            ),
                            _ap(sap, C, [sap.ap[0], [128, 4], [1, C]]),
                        )

            # ================= Phase B: per-patch sample ===================
            with (
                tc.tile_pool(name="crd", bufs=2) as crd,
                tc.tile_pool(name="gpool", bufs=2) as gpool,
                tc.tile_pool(name="fpool", bufs=2) as fpool,
                tc.tile_pool(name="tpsum", bufs=4, space="PSUM") as tpsum,
                tc.tile_pool(name="opool", bufs=2) as opool,
            ):
                for n in range(NP):
                    cn = cos_bc[:, n : n + 1]
                    sn = sin_bc[:, n : n + 1]
                    un = u_bc[:, n : n + 1]
                    vn = v_bc[:, n : n + 1]

                    # grid_u = (u + cos*gu0) - sin*gv0
                    # grid_v = (v + sin*gu0) + cos*gv0
                    xu = crd.tile([128, 32], F32)
                    nc.vector.tensor_scalar(xu[:], gu0[:], cn, un, ALU.mult, ALU.add)
                    t3 = crd.tile([128, 1], F32)
                    nc.vector.tensor_tensor(t3[:], gv0[:], sn, ALU.mult)
                    nc.vector.tensor_scalar(xu[:], xu[:], t3[:, 0:1], None, ALU.subtract)

                    yv = crd.tile([128, 32], F32)
                    nc.vector.tensor_scalar(yv[:], gu0[:], sn, vn, ALU.mult, ALU.add)
                    s3 = crd.tile([128, 1], F32)
                    nc.vector.tensor_tensor(s3[:], gv0[:], cn, ALU.mult)
                    nc.vector.tensor_scalar(yv[:], yv[:], s3[:, 0:1], None, ALU.add)

                    # g = (coord + 0.5) / 256 - 1 ; valid = |g| < 1
                    # coord' = ((g + 1) * 256) - 0.5   (exact ref rounding chain)
                    def coord_chain(src_t, dst_x, dst_v):
                        a = crd.tile([128, 32], F32, tag="cc_a")
                        nc.vector.tensor_scalar(
                            a[:], src_t[:], 0.5, 0.00390625, ALU.add, ALU.mult
                        )
                        g = crd.tile([128, 32], F32, tag="cc_g")
                        nc.vector.tensor_scalar(g[:], a[:], -1.0, None, ALU.add)
                        ag = crd.tile([128, 32], F32, tag="cc_ag")
                        nc.scalar.activation(ag[:], g[:], ACTF.Abs, bias=zbias[:, 0:1])
                        nc.vector.tensor_scalar(dst_v[:], ag[:], 1.0, None, ALU.is_lt)
                        b1 = crd.tile([128, 32], F32, tag="cc_b")
                        nc.vector.tensor_scalar(
                            b1[:], g[:], 1.0, 256.0, ALU.add, ALU.mult
                        )
                        nc.vector.tensor_scalar(dst_x[:], b1[:], -0.5, None, ALU.add)

                    x = crd.tile([128, 32], F32)
                    yy = crd.tile([128, 32], F32)
                    vx = crd.tile([128, 32], F32)
                    vy = crd.tile([128, 32], F32)
                    coord_chain(xu, x, vx)
                    coord_chain(yv, yy, vy)
                    valid = crd.tile([128, 32], F32)
                    nc.vector.tensor_tensor(valid[:], vx[:], vy[:], ALU.mult)

                    # floor via RNE(x - 0.5) with the 1.5*2^23 trick
                    def floor_frac(src_t, dst_f, dst_w):
                        m = crd.tile([128, 32], F32, tag="ff_m")
                        nc.vector.tensor_scalar(
                            m[:], src_t[:], -0.5, MAGIC, ALU.add, ALU.add
                        )
                        nc.vector.tensor_scalar(dst_f[:], m[:], -MAGIC, None, ALU.add)
                        nc.vector.tensor_tensor(
                            dst_w[:], src_t[:], dst_f[:], ALU.subtract
                        )

                    x0f = crd.tile([128, 32], F32)
                    wx = crd.tile([128, 32], F32)
                    y0f = crd.tile([128, 32], F32)
                    wy = crd.tile([128, 32], F32)
                    floor_frac(x, x0f, wx)
                    floor_frac(yy, y0f, wy)

                    # weights (valid folded into the y-terms)
                    wyb = crd.tile([128, 32], F32)
                    nc.vector.tensor_scalar(wyb[:], wy[:], -1.0, 1.0, ALU.mult, ALU.add)
                    wA = crd.tile([128, 32], F32)
                    nc.vector.tensor_tensor(wA[:], wyb[:], valid[:], ALU.mult)
                    wB = crd.tile([128, 32], F32)
                    nc.vector.tensor_tensor(wB[:], wy[:], valid[:], ALU.mult)
                    wxb = crd.tile([128, 32], F32)
                    nc.vector.tensor_scalar(wxb[:], wx[:], -1.0, 1.0, ALU.mult, ALU.add)
                    wtl = crd.tile([128, 32], F32)
                    wtr = crd.tile([128, 32], F32)
                    wbl = crd.tile([128, 32], F32)
                    wbr = crd.tile([128, 32], F32)
                    nc.vector.tensor_tensor(wtl[:], wxb[:], wA[:], ALU.mult)
                    nc.vector.tensor_tensor(wtr[:], wx[:], wA[:], ALU.mult)
                    nc.vector.tensor_tensor(wbl[:], wxb[:], wB[:], ALU.mult)
                    nc.vector.tensor_tensor(wbr[:], wx[:], wB[:], ALU.mult)

                    # gather slot index: ((y0+1)*Wp + (x0+1)) * valid
                    sx = crd.tile([128, 32], F32)
                    nc.vector.tensor_scalar(sx[:], x0f[:], 1.0, None, ALU.add)
                    ry = crd.tile([128, 32], F32)
                    nc.vector.tensor_scalar(ry[:], y0f[:], 1.0, None, ALU.add)
                    idxr = crd.tile([128, 32], F32)
                    nc.vector.scalar_tensor_tensor(
                        idxr[:], ry[:], float(Wp), sx[:], ALU.mult, ALU.add
                    )
                    topf = crd.tile([128, 32], F32)
                    nc.vector.tensor_tensor(topf[:], idxr[:], valid[:], ALU.mult)
                    it32 = crd.tile([128, 32], I32)
                    nc.vector.tensor_copy(it32[:], topf[:])

                    # gathers: one index per partition; each reads 256
                    # contiguous floats = q slots (y,x0),(y,x0+1) = 4 corners
                    ga = gpool.tile([128, 32 * 256], F32, tag="ga")
                    gap = ga[:]
                    for g in range(32):
                        nc.gpsimd.indirect_dma_start(
                            out=_ap(gap, g * 256, [gap.ap[0], [1, 256]]),
                            out_offset=None,
                            in_=qimg,
                            in_offset=bass.IndirectOffsetOnAxis(
                                ap=it32[:, g : g + 1], axis=0
                            ),
                        )

                    # combine: F[p, if, c] = tl*wtl + tr*wtr + bl*wbl + br*wbr
                    # gathered layout per if: [0:64]=(x0,y0) [64:128]=(x0,y1)
                    #                         [128:192]=(x1,y0) [192:256]=(x1,y1)
                    ft = fpool.tile([128, 2048], F32, tag="ft")
                    f3 = _ap(ft[:], 0, [ft[:].ap[0], [64, 32], [1, 64]])

                    def corner(off):
                        return _ap(gap, off, [gap.ap[0], [256, 32], [1, 64]])

                    def wb_(w_t):
                        wap = w_t[:]
                        return _ap(wap, 0, [wap.ap[0], [1, 32], [0, 64]])

                    tmp = fpool.tile([128, 2048], F32, tag="tmp")
                    t3v = _ap(tmp[:], 0, [tmp[:].ap[0], [64, 32], [1, 64]])
                    nc.vector.tensor_tensor(f3, corner(0), wb_(wtl), ALU.mult)
                    nc.vector.tensor_tensor(t3v, corner(128), wb_(wtr), ALU.mult)
                    nc.vector.tensor_tensor(f3, f3, t3v, ALU.add)
                    nc.vector.tensor_tensor(t3v, corner(64), wb_(wbl), ALU.mult)
                    nc.vector.tensor_tensor(f3, f3, t3v, ALU.add)
                    nc.vector.tensor_tensor(t3v, corner(192), wb_(wbr), ALU.mult)
                    nc.vector.tensor_tensor(f3, f3, t3v, ALU.add)

                    # transpose to channel-major and store
                    ot = opool.tile([128, 2048], F32)
                    for g in range(4):
                        pt = tpsum.tile([128, 512], F32)
                        for q in range(4):
                            t = 4 * g + q
                            nc.tensor.transpose(
                                out=pt[:, 128 * q : 128 * (q + 1)],
                                in_=ft[:, 128 * t : 128 * (t + 1)],
                                identity=ident[:],
                            )
                        nc.scalar.copy(ot[:, 512 * g : 512 * (g + 1)], pt[:])

                    dst = _ap(
                        out,
                        n * C * PIX,
                        [[128, 2], [PIX, 64], [256, 16], [1, 128]],
                    )
                    nc.sync.dma_start(dst, ot[:])

    nc.compile()
    return nc


_NC_CACHE = None


def _get_nc():
    global _NC_CACHE
    if _NC_CACHE is None:
        _NC_CACHE = build_program()
    return _NC_CACHE


def make_in_maps(aer_feat, pose_uvr, offset_scale):
    in_maps = []
    for k in range(NCORES):
        b = k // (NCORES // B)
        n0 = (k % (NCORES // B)) * NP
        in_maps.append(
            {
                "img": np.ascontiguousarray(aer_feat[b]),
                "pose": np.ascontiguousarray(pose_uvr[b, n0 : n0 + NP]),
                "osc": np.ascontiguousarray(
                    offset_scale[b].reshape(1, 1).astype(np.float32)
                ),
            }
        )
    return in_maps


def assemble(results):
    full = np.empty((B, N, C, HB, WB), dtype=np.float32)
    for k in range(NCORES):
        b = k // (NCORES // B)
        n0 = (k % (NCORES // B)) * NP
        full[b, n0 : n0 + NP] = results[k]["out"].reshape(NP, C, HB, WB)
    return full


def kernel(aer_feat, pose_uvr, offset_scale):
    from concourse.bass_utils import run_bass_kernel_spmd

    nc = _get_nc()
    in_maps = make_in_maps(aer_feat, pose_uvr, offset_scale)
    res = run_bass_kernel_spmd(nc, in_maps, list(range(NCORES)))
    return assemble(res.results)
